# revision 110
# baseline (speedup 1.0000x reference)
"""Trainium2 Bass kernel for CliffordFrameAttention (v3, head-outer pipelined).

Sharding: 8 cores = 2 batches x 4 head-pairs; each core computes two full
attention heads (L=2048) for one batch element plus the fused Clifford
geometric products, emitting a per-core partial [16, 128, 32] output that
the host sums per batch.

Algebra (same as validated baseline):
  - deferred softmax: P = exp(S) * mask, rowsum rs via ones-column in the
    PV matmul, final scale by 1/rs at the end.
  - gp_qv + 0.25*gp_qk + V_agg collapses to gp(Q, U) + U@Wo^T with
    U = PV + 0.25*rs*K, minus a 0.25*x@W2sum correction.
  - Wo folded into the Cayley tensor so heads sum on the host.

Performance structure (cost-model time 146 us/core vs 247 us baseline):
  - whole mask resident in SBUF, loaded via 16 single-chunk DMAs
    alternating the Pool/SP queues so arrival cadence beats phase B's
    consumption; DMA spans hold their queue end-to-end so the Act queue
    issues no DMAs.
  - head-outer loop with software pipelining: S(c+1)+exp(c+1) are emitted
    before mask/PV(c) so the PE queue never parks an S matmul behind a PV
    that waits on the DVE mask multiply.
  - head 0's entire tail (rs transpose, U correction, Q/U replication,
    T tiles, geometric product) is emitted as 20 steps interleaved into
    head 1's main phase; head 0's Q replication rides Pool broadcast DMAs
    from a DRAM bounce, while head 1's (in the un-hideable final tail)
    uses PE selector-matmul replication + Act psum->sbuf copies, both
    engines otherwise idle there.
  - geometric-product accumulators are carved out of ps_vu PSUM
    partitions freed by the U copy, so the S double buffer keeps both
    PSUM work buffers during the overlap.
  - elementwise multiplies ride DVE 2x bf16 mode; psum->sbuf copies are
    balanced between Act and DVE according to which phase has slack.
  - matmul PSUM outputs never exceed 512 f32 columns (one bank), a hard
    walrus/ISA constraint.
"""

import math
import os
import sys

for _p in ("/opt/trn_rl_repo", "/opt/trn_rl_repo/concourse"):
    if _p not in sys.path:
        sys.path.insert(0, _p)

import numpy as np
import ml_dtypes

import concourse.bass as bass
import concourse.mybir as mybir
import concourse.tile as tile
from concourse import bacc
from concourse.bass_utils import run_bass_kernel_spmd

BF16 = ml_dtypes.bfloat16
F32 = mybir.dt.float32
F32R = mybir.dt.float32r
BF = mybir.dt.bfloat16

N_CORES = 8
B, L, D = 2, 2048, 32
H = 8
NC16 = 16

_compiled_nc = None
LAST_RESULT = None


def _build():
    nc = bacc.Bacc("TRN2", target_bir_lowering=False, debug=False,
                   num_devices=N_CORES)

    xT_d = nc.declare_dram_parameter("xT", [32, L], F32R, isOutput=False)
    maskT_d = nc.declare_dram_parameter("maskT", [L, L], BF, isOutput=False)
    wcat_d = nc.declare_dram_parameter("wcat", [32, 384], F32R, isOutput=False)
    cp2_d = nc.declare_dram_parameter("cp2", [128, 512], BF, isOutput=False)
    wex_d = nc.declare_dram_parameter("wex", [128, 384], BF, isOutput=False)
    rsel_d = nc.declare_dram_parameter("rsel", [32, 1024], BF, isOutput=False)
    out_d = nc.declare_dram_parameter("out", [NC16, 128, 32], F32, isOutput=True)

    rs_dram = nc.dram_tensor("rs_bounce", [2, L], F32)
    qT_dram = nc.dram_tensor("qT_bounce", [2, 32, L], BF)
    uT_dram = nc.dram_tensor("uT_bounce", [2, 32, L], BF)

    with tile.TileContext(nc) as tc:
        with (
            tc.tile_pool(name="const", bufs=1) as cpool,
            tc.tile_pool(name="pt", bufs=5) as ptpool,
            tc.tile_pool(name="qrepa", bufs=1) as qrpoolA,
            tc.tile_pool(name="qhb", bufs=3) as qhpool,
            tc.tile_pool(name="urep", bufs=1) as urpool,
            tc.tile_pool(name="tbuf", bufs=3) as tpool,
            tc.tile_pool(name="small", bufs=2) as spool,
            tc.tile_pool(name="gpin", bufs=2) as gpool,
            tc.tile_pool(name="ps", bufs=1, space="PSUM") as pspool,
            tc.tile_pool(name="psw", bufs=2, space="PSUM") as pswork,
        ):
            # ---------- constants / inputs ----------
            xT = cpool.tile([32, L], F32R, tag="xT")
            nc.sync.dma_start(out=xT[:], in_=xT_d[:])
            wcat = cpool.tile([32, 384], F32R, tag="wcat")
            nc.sync.dma_start(out=wcat[:], in_=wcat_d[:])

            # whole mask resident; DMA spans hold their queue end-to-end, so
            # pairs alternate Pool/SP ordered to arrive just before phase B
            # consumes them. Act issues no DMAs (it paces the exp stream).
            maskT_sb = cpool.tile([128, NC16, L], BF, tag="mask")

            def load_mask_chunk(c):
                eng = nc.gpsimd if c % 2 == 0 else nc.sync
                eng.dma_start(
                    out=maskT_sb[:, c, :],
                    in_=maskT_d[128 * c:128 * c + 128, :],
                )

            for _c in range(4):
                load_mask_chunk(_c)
            cp_sb = cpool.tile([128, 8, 64], BF, tag="cp")
            wex = cpool.tile([128, 384], BF, tag="wex")
            woT2 = [wex[0:32, 0:32], wex[0:32, 192:224]]
            id25 = wex[:, 32:160]
            id1 = wex[0:32, 160:192]
            r4 = wex[0:32, 224:352]    # [I32 I32 I32 I32] selector

            qku = cpool.tile([32, 4 * L], F32R, tag="qku")
            qbf1 = cpool.tile([32, L], BF, tag="qbf1")   # h1 Q^T bf16
            ubf1 = cpool.tile([32, L], BF, tag="ubf1")   # h1 U bf16
            rsel = cpool.tile([32, 8, 128], BF, tag="rsel")
            proj_l = cpool.tile([128, NC16, 164], BF, tag="projl")
            rs_lp = cpool.tile([128, 32], F32, tag="rslp")
            invrs = cpool.tile([128, 32], F32, tag="invrs")
            nv25 = cpool.tile([128, 16], F32, tag="nv25")
            w2g = cpool.tile([128, NC16, 32], BF, tag="w2g")
            final_sb = cpool.tile([128, NC16, 32], F32, tag="final")

            nc.gpsimd.memset(proj_l[:, :, 32:33], 1.0)
            nc.gpsimd.memset(proj_l[:, :, 65:66], 1.0)

            # ---------- phase A helpers ----------
            # head h's Q/Kg projections. Head 0's qku copies ride Act (fast,
            # pipeline-critical); head 1's (emitted interleaved into B(h0))
            # and the bf16 qT8 copies ride the DVE.
            def emit_qk_proj(h, t, lh):
                ps_qk = pswork.tile([128, 1024], F32, tag="work")
                for nt in range(2):
                    nc.tensor.matmul(
                        ps_qk[0:32, 512 * nt:512 * nt + 512],
                        wcat[:, 64 * h + 32 * t:64 * h + 32 * t + 32],
                        xT[:, 1024 * lh + 512 * nt:1024 * lh + 512 * nt + 512],
                        start=True, stop=True,
                    )
                qsl = qku[:, L * (2 * h + t) + 1024 * lh:
                          L * (2 * h + t) + 1024 * lh + 1024]
                if h == 0 and t == 0:
                    nc.scalar.copy(out=qsl, in_=ps_qk[0:32, :])
                else:
                    nc.vector.tensor_copy(out=qsl, in_=ps_qk[0:32, :])

            def emit_qT8(h):
                # bf16 Q^T staging: head 0 bounces via DRAM for the broadcast
                # DMA; head 1 keeps it in SBUF for PE-selector replication.
                if h == 1:
                    nc.vector.tensor_copy(out=qbf1[:], in_=qku[:, 2 * L:3 * L])
                    return
                for lh in range(2):
                    qT8h = gpool.tile([32, 1024], BF, tag="gpin")
                    nc.vector.tensor_copy(
                        out=qT8h[:],
                        in_=qku[:, 2 * L * h + 1024 * lh:
                                2 * L * h + 1024 * lh + 1024])
                    nc.gpsimd.dma_start(
                        out=qT_dram[h][:, 1024 * lh:1024 * lh + 1024],
                        in_=qT8h[:])

            def emit_vk_proj(c, on_act=False):
                ps_vk = pswork.tile([128, 1024], F32, tag="work")
                nc.tensor.matmul(
                    ps_vk[:, 0:256],
                    xT[:, 128 * c:128 * c + 128],
                    wcat[:, 128:384],
                    start=True, stop=True,
                )
                vdst = proj_l[:, c, 0:66].rearrange("p (a b) -> p a b", a=2)[:, :, 0:32]
                vsrc = ps_vk[:, 0:64].rearrange("p (a b) -> p a b", a=2)
                if on_act:
                    nc.scalar.copy(out=vdst, in_=vsrc)
                    nc.scalar.copy(out=proj_l[:, c, 66:162], in_=ps_vk[:, 64:160])
                else:
                    nc.vector.tensor_copy(out=vdst, in_=vsrc)
                    nc.vector.tensor_copy(out=proj_l[:, c, 66:162],
                                          in_=ps_vk[:, 64:160])

            def load_qrep(h, pool, engines):
                """Full [128, 8, L] replication tile for head h."""
                qh = pool.tile([128, 8, L], BF, tag="qrep")
                qv4 = qT_dram[h].rearrange("(a i) l -> i a l", a=8)
                for i in range(4):
                    engines[i % len(engines)].dma_start(
                        out=qh[32 * i:32 * i + 32, :, :],
                        in_=qv4[i].unsqueeze(0).to_broadcast([32, 8, L]),
                    )
                return qh

            # ---------- phase A: head 0 projections only ----------
            for lh in range(2):
                for t in range(2):
                    emit_qk_proj(0, t, lh)
            emit_qT8(0)
            for _c in range(4, 8):
                load_mask_chunk(_c)
            for c in range(3):
                emit_vk_proj(c, on_act=True)

            ps_vu = pspool.tile([128, L], F32, tag="vu")

            # ---------- emission helpers ----------
            # S+exp for chunk c and mask+PV for chunk c-1 are emitted
            # staggered so the PE queue never parks an S matmul behind a PV
            # that waits on the DVE mask multiply (dependency-cycle pacing).
            def emit_s_exp(h, c):
                qw = 2 * L * h
                pt = ptpool.tile([128, L], BF, tag="pt")
                for lh in range(2):
                    ps_s = pswork.tile([128, 1024], F32, tag="work")
                    for nt in range(2):
                        nc.tensor.matmul(
                            ps_s[:, 512 * nt:512 * nt + 512],
                            qku[:, qw + L + 128 * c:qw + L + 128 * c + 128],
                            qku[:, qw + 1024 * lh + 512 * nt:
                                qw + 1024 * lh + 512 * nt + 512],
                            start=True, stop=True,
                        )
                    nc.scalar.activation(
                        pt[:, 1024 * lh:1024 * lh + 1024], ps_s[:],
                        mybir.ActivationFunctionType.Exp,
                    )
                return pt

            def emit_warm(n):
                # Keep-warm matmuls into unused ps_vu partitions 33..63: the
                # PE clock ramps to full speed only after ~3us of continuous
                # busy, so bridging its sub-us idle gaps doubles S/PV speed.
                # scratch region: h1's PV rows, free for the whole of B(h0);
                # PV(h1, c0)'s start=True reset clears the garbage.
                for i in range(n):
                    nc.tensor.matmul(
                        ps_vu[64:94, 512 * (i % 4):512 * (i % 4) + 512],
                        id25[0:32, 0:30], maskT_sb[0:32, 0, 0:512],
                        start=True, stop=True, skip_group_check=True,
                    )

            def emit_mask_pv(h, c, pt, warm=0):
                nc.vector.tensor_tensor(
                    out=pt[:], in0=pt[:], in1=maskT_sb[:, c, :],
                    op=mybir.AluOpType.mult,
                )
                for nt in range(4):
                    nc.tensor.matmul(
                        ps_vu[64 * h:64 * h + 33, 512 * nt:512 * nt + 512],
                        proj_l[:, c, 33 * h:33 * h + 33],
                        pt[:, 512 * nt:512 * nt + 512],
                        start=(c == 0), stop=(c == NC16 - 1),
                        skip_group_check=True,
                    )
                emit_warm(warm)

            def phase_c_steps(h, qrep):
                """Emit head-h tail as a list of thunks (interleavable)."""
                v0 = 64 * h
                state = {}

                def s_rs_half(lh):
                    def f():
                        rs_seq = ptpool.tile([1, 1024], F32, tag="pt")
                        nc.scalar.copy(
                            out=rs_seq[:],
                            in_=ps_vu[v0 + 32:v0 + 33, 1024 * lh:1024 * lh + 1024])
                        nc.sync.dma_start(
                            out=rs_dram[h][1024 * lh:1024 * lh + 1024].unsqueeze(0),
                            in_=rs_seq[:, :])
                        col = 16 * h + 8 * lh
                        nc.sync.dma_start(
                            out=rs_lp[:, col:col + 8],
                            in_=rs_dram[h][1024 * lh:1024 * lh + 1024]
                                .rearrange("(c p) -> p c", p=128),
                        )
                        tmp16 = spool.tile([128, 8], F32, tag="tmp16")
                        nc.vector.tensor_scalar(tmp16[:], rs_lp[:, col:col + 8],
                                                1e-30, None,
                                                op0=mybir.AluOpType.add)
                        nc.vector.reciprocal(invrs[:, col:col + 8], tmp16[:])
                        if h == 0:
                            nc.vector.tensor_scalar(
                                nv25[:, 8 * lh:8 * lh + 8], rs_lp[:, col:col + 8],
                                0.0, -0.25, op0=mybir.AluOpType.is_gt,
                                op1=mybir.AluOpType.mult)
                            for gl in range(8 * lh, 8 * lh + 8):
                                nc.vector.tensor_scalar(
                                    w2g[:, gl, :], proj_l[:, gl, 130:162],
                                    nv25[:, gl:gl + 1], None,
                                    op0=mybir.AluOpType.mult)
                    return f

                def s_ucorr(lh):
                    def f():
                        for c in range(8 * lh, 8 * lh + 8):
                            kwin = proj_l[:, c, 66 + 32 * h:98 + 32 * h]
                            nc.vector.tensor_scalar(
                                kwin, kwin,
                                rs_lp[:, 16 * h + c:16 * h + c + 1], None,
                                op0=mybir.AluOpType.mult)
                            nc.tensor.matmul(
                                ps_vu[v0:v0 + 32, 128 * c:128 * c + 128],
                                kwin, id25,
                                start=False, stop=True, skip_group_check=True,
                            )
                    return f

                def s_uv(lh):
                    # h0: U lands in row-group 0 of its replication tile and
                    # bounces via DRAM (overlapped under B(h1)); h1: U goes to
                    # a bf16 strip for PE-selector replication in the tail.
                    def f():
                        if "urep" not in state:
                            urep_t = urpool.tile([128, L], BF, tag="urep")
                            state["urep"] = urep_t
                        urep = state["urep"]
                        sl = slice(1024 * lh, 1024 * lh + 1024)
                        if h == 0:
                            nc.vector.tensor_copy(out=urep[0:32, sl],
                                                  in_=ps_vu[v0:v0 + 32, sl])
                            nc.sync.dma_start(out=uT_dram[h][:, sl],
                                              in_=urep[0:32, sl])
                        else:
                            nc.vector.tensor_copy(out=ubf1[:, sl],
                                                  in_=ps_vu[v0:v0 + 32, sl])
                    return f

                def s_urep(lh, engines):
                    def f():
                        urep = state["urep"]
                        sl = slice(1024 * lh, 1024 * lh + 1024)
                        if h == 0:
                            for r in range(1, 4):
                                engines[r % len(engines)].dma_start(
                                    out=urep[32 * r:32 * r + 32, sl],
                                    in_=uT_dram[h][:, sl],
                                )
                        else:
                            ps_u = pswork.tile([128, 1024], F32, tag="work")
                            for nt in range(2):
                                nc.tensor.matmul(
                                    ps_u[:, 512 * nt:512 * nt + 512], r4,
                                    ubf1[:, 1024 * lh + 512 * nt:
                                         1024 * lh + 512 * nt + 512],
                                    start=True, stop=True)
                            nc.scalar.copy(out=urep[:, sl], in_=ps_u[:])
                    return f

                def s_gpstart(hf):
                    # gp accumulators live in ps_vu rows [v0, v0+32) freed by s_uv
                    def f():
                        usrc = state["urep"] if h == 0 else ubf1
                        for nt in range(2):
                            sl = slice(1024 * hf + 512 * nt,
                                       1024 * hf + 512 * nt + 512)
                            nc.tensor.matmul(
                                ps_vu[v0:v0 + 32, sl],
                                woT2[h], usrc[0:32, sl],
                                start=True, stop=False, skip_group_check=True,
                            )
                    return f

                def s_ta(a, halves=(0, 1)):
                    def f():
                        urep = state["urep"]
                        for hf in halves:
                            if qrep is not None:
                                qh = qrep[:, a, 1024 * hf:1024 * hf + 1024]
                            else:
                                # PE selector replication; alternate units
                                # between an Act psum->sbuf copy and a direct
                                # PSUM read in the DVE multiply so neither
                                # engine paces the whole stream
                                ps_q = pswork.tile([128, 1024], F32, tag="work")
                                for nt in range(2):
                                    nc.tensor.matmul(
                                        ps_q[:, 512 * nt:512 * nt + 512],
                                        rsel[:, a, :],
                                        qbf1[:, 1024 * hf + 512 * nt:
                                             1024 * hf + 512 * nt + 512],
                                        start=True, stop=True,
                                    )
                                qh_t = qhpool.tile([128, 1024], BF, tag="qhb")
                                nc.scalar.copy(out=qh_t[:], in_=ps_q[:])
                                qh = qh_t[:]
                            th = tpool.tile([128, 1024], BF, tag="tt")
                            nc.vector.tensor_tensor(
                                out=th[:], in0=qh,
                                in1=urep[:, 1024 * hf:1024 * hf + 1024],
                                op=mybir.AluOpType.mult)
                            for nt in range(2):
                                sl = slice(1024 * hf + 512 * nt,
                                           1024 * hf + 512 * nt + 512)
                                nc.tensor.matmul(
                                    ps_vu[v0:v0 + 32, sl],
                                    cp_sb[:, a, 32 * h:32 * h + 32],
                                    th[:, 512 * nt:512 * nt + 512],
                                    start=False, stop=(a == 7),
                                    skip_group_check=True,
                                )
                    return f

                def s_tail(hf):
                    def f():
                        gp_in = gpool.tile([32, 1024], BF, tag="gpin")
                        nc.vector.tensor_copy(
                            out=gp_in[:],
                            in_=ps_vu[v0:v0 + 32, 1024 * hf:1024 * hf + 1024])
                        ps_tr = pswork.tile([128, 256], BF, tag="work")
                        for lt in range(8):
                            nc.tensor.transpose(
                                out=ps_tr[:, 32 * lt:32 * lt + 32],
                                in_=gp_in[:, 128 * lt:128 * lt + 128],
                                identity=id1,
                            )
                        for lt in range(8):
                            gl = 8 * hf + lt
                            in1 = w2g[:, gl, :] if h == 0 else final_sb[:, gl, :]
                            nc.vector.scalar_tensor_tensor(
                                out=final_sb[:, gl, :],
                                in0=ps_tr[:, 32 * lt:32 * lt + 32],
                                scalar=invrs[:, 16 * h + gl:16 * h + gl + 1],
                                in1=in1,
                                op0=mybir.AluOpType.mult,
                                op1=mybir.AluOpType.add)
                    return f

                ur_engines = [nc.sync] if h == 0 else [nc.sync, nc.gpsimd]
                if h == 0:
                    steps = [s_rs_half(0), s_rs_half(1),
                             s_ucorr(0), s_uv(0), s_urep(0, ur_engines),
                             s_gpstart(0),
                             s_ucorr(1), s_uv(1), s_urep(1, ur_engines),
                             s_gpstart(1)]
                    steps += [s_ta(a) for a in range(8)]
                    steps += [s_tail(0), s_tail(1)]
                else:
                    steps = [s_rs_half(0), s_rs_half(1),
                             s_ucorr(0), s_uv(0), s_urep(0, ur_engines),
                             s_gpstart(0),
                             s_ucorr(1), s_uv(1), s_urep(1, ur_engines),
                             s_gpstart(1)]
                    steps += [s_ta(a) for a in range(8)]
                    steps += [s_tail(0), s_tail(1)]
                return steps

            # ---------- head 0 main, with interleaved h1 projections ----------
            qrep0 = qrep1 = None
            h1_proj_units = [(t, lh) for lh in range(2) for t in range(2)]
            pt_prev = None
            for c in range(NC16):
                pt_c = emit_s_exp(0, c)
                if pt_prev is not None:
                    emit_mask_pv(0, c - 1, pt_prev, warm=0)
                pt_prev = pt_c
                if c <= 3:
                    load_mask_chunk(8 + 2 * c)
                    load_mask_chunk(9 + 2 * c)
                if c in (0, 2, 4, 6):
                    t, lh = h1_proj_units[c // 2]
                    emit_qk_proj(1, t, lh)
                elif c == 7:
                    emit_qT8(1)
                elif c == 5:
                    nc.sync.dma_start(
                        out=cp_sb[:].rearrange("p a c -> p (a c)"), in_=cp2_d[:])
                    nc.sync.dma_start(out=wex[:], in_=wex_d[:])
                elif c == 8:
                    qrep0 = load_qrep(0, qrpoolA, [nc.gpsimd])
                elif c == 9:
                    nc.sync.dma_start(
                        out=rsel[:].rearrange("p a c -> p (a c)"), in_=rsel_d[:])
                if 2 <= c < 15:
                    emit_vk_proj(c + 1, on_act=True)
            emit_mask_pv(0, NC16 - 1, pt_prev)

            # ---------- head 0 tail interleaved with head 1 main ----------
            steps0 = phase_c_steps(0, qrep0)
            pt_prev = None
            for c in range(NC16):
                pt_c = emit_s_exp(1, c)
                if pt_prev is not None:
                    emit_mask_pv(1, c - 1, pt_prev)
                pt_prev = pt_c
                if c < len(steps0):
                    steps0[c]()
            emit_mask_pv(1, NC16 - 1, pt_prev)
            for s in steps0[NC16:]:
                s()

            # ---------- head 1 tail ----------
            for s in phase_c_steps(1, None):
                s()
            nc.sync.dma_start(
                out=out_d[:].rearrange("c p d -> p c d"), in_=final_sb[:])

    nc.compile()
    return nc


def _get_nc():
    global _compiled_nc
    if _compiled_nc is None:
        _compiled_nc = _build()
    return _compiled_nc


def _in_maps(x, mask, Wq, Wk, Wv, Wo, cayley, gs):
    s = 1.0 / math.sqrt(D)
    in_maps = []
    for core in range(N_CORES):
        b, hp = core // 4, core % 4
        heads = (2 * hp, 2 * hp + 1)
        xT = np.ascontiguousarray(x[b].T)
        maskT = np.ascontiguousarray(mask[b].T).astype(BF16)

        wcat = np.zeros((32, 384), np.float32)
        cp = np.zeros((1024, 64), np.float32)
        wex = np.zeros((128, 384), np.float32)
        W2sum = np.zeros((32, 32), np.float32)
        for j, h in enumerate(heads):
            Wq_h = Wq[32 * h:32 * h + 32]
            Wk_h = Wk[32 * h:32 * h + 32]
            Wv_h = Wv[32 * h:32 * h + 32]
            Wo_h = Wo[:, 32 * h:32 * h + 32]
            wcat[:, 64 * j:64 * j + 32] = Wq_h.T * s
            wcat[:, 64 * j + 32:64 * j + 64] = Wk_h.T * gs[None, :]
            wcat[:, 128 + 32 * j:128 + 32 * j + 32] = Wv_h.T
            wcat[:, 192 + 32 * j:192 + 32 * j + 32] = Wk_h.T
            W2sum += Wk_h.T @ Wo_h.T
            cp[:, 32 * j:32 * j + 32] = (
                math.sqrt(D) * np.einsum('ijk,dk->ijd', cayley, Wo_h)
            ).reshape(1024, 32)
            wex[0:32, (0 if j == 0 else 192):(32 if j == 0 else 224)] = Wo_h.T
        wcat[:, 256:288] = W2sum
        wex[:, 32:160] = 0.25 * np.eye(128)
        wex[0:32, 160:192] = np.eye(32)
        wex[0:32, 224:352] = np.tile(np.eye(32), (1, 4))
        cp2 = np.ascontiguousarray(
            cp.reshape(8, 128, 64).transpose(1, 0, 2).reshape(128, 512))

        # selector for PE replication: rsel[q, 128a + 32i + j] = (q == 4a+i)
        rsel = np.zeros((32, 8, 4, 32), np.float32)
        for a in range(8):
            for i in range(4):
                rsel[4 * a + i, a, i, :] = 1.0
        rsel = rsel.reshape(32, 1024)

        in_maps.append({
            "xT": xT,
            "maskT": maskT,
            "wcat": wcat,
            "cp2": cp2.astype(BF16),
            "wex": wex.astype(BF16),
            "rsel": rsel.astype(BF16),
        })
    return in_maps


def kernel(x, mask, Wq, Wk, Wv, Wo, cayley, grade_signs):
    x = np.asarray(x, dtype=np.float32)
    mask = np.asarray(mask)
    Wq = np.asarray(Wq, dtype=np.float32)
    Wk = np.asarray(Wk, dtype=np.float32)
    Wv = np.asarray(Wv, dtype=np.float32)
    Wo = np.asarray(Wo, dtype=np.float32)
    cayley = np.asarray(cayley, dtype=np.float32)
    gs = np.asarray(grade_signs, dtype=np.float32)

    in_maps = _in_maps(x, mask, Wq, Wk, Wv, Wo, cayley, gs)

    _trace = bool(os.environ.get("KTRACE"))
    res = run_bass_kernel_spmd(_get_nc(), in_maps, list(range(N_CORES)),
                               trace=_trace)
    global LAST_RESULT
    LAST_RESULT = res
    out = np.zeros((B, L, D), np.float32)
    for core in range(N_CORES):
        out[core // 4] += res.results[core]["out"].reshape(L, 32)
    return out


# revision 117
# speedup vs baseline: 1.0131x; 1.0131x over previous
"""Trainium2 Bass kernel for CliffordFrameAttention (v3, head-outer pipelined).

Sharding: 8 cores = 2 batches x 4 head-pairs; each core computes two full
attention heads (L=2048) for one batch element plus the fused Clifford
geometric products, emitting a per-core partial [16, 128, 32] output that
the host sums per batch.

Algebra (same as validated baseline):
  - deferred softmax: P = exp(S) * mask, rowsum rs via ones-column in the
    PV matmul, final scale by 1/rs at the end.
  - gp_qv + 0.25*gp_qk + V_agg collapses to gp(Q, U) + U@Wo^T with
    U = PV + 0.25*rs*K, minus a 0.25*x@W2sum correction.
  - Wo folded into the Cayley tensor so heads sum on the host.

Performance structure (cost-model time 146 us/core vs 247 us baseline):
  - whole mask resident in SBUF, loaded via 16 single-chunk DMAs
    alternating the Pool/SP queues so arrival cadence beats phase B's
    consumption; DMA spans hold their queue end-to-end so the Act queue
    issues no DMAs.
  - head-outer loop with software pipelining: S(c+1)+exp(c+1) are emitted
    before mask/PV(c) so the PE queue never parks an S matmul behind a PV
    that waits on the DVE mask multiply.
  - head 0's entire tail (rs transpose, U correction, Q/U replication,
    T tiles, geometric product) is emitted as 20 steps interleaved into
    head 1's main phase; head 0's Q replication rides Pool broadcast DMAs
    from a DRAM bounce, while head 1's (in the un-hideable final tail)
    uses PE selector-matmul replication + Act psum->sbuf copies, both
    engines otherwise idle there.
  - geometric-product accumulators are carved out of ps_vu PSUM
    partitions freed by the U copy, so the S double buffer keeps both
    PSUM work buffers during the overlap.
  - elementwise multiplies ride DVE 2x bf16 mode; psum->sbuf copies are
    balanced between Act and DVE according to which phase has slack.
  - matmul PSUM outputs never exceed 512 f32 columns (one bank), a hard
    walrus/ISA constraint.
"""

import math
import os
import sys

for _p in ("/opt/trn_rl_repo", "/opt/trn_rl_repo/concourse"):
    if _p not in sys.path:
        sys.path.insert(0, _p)

import numpy as np
import ml_dtypes

import concourse.bass as bass
import concourse.mybir as mybir
import concourse.tile as tile
from concourse import bacc
from concourse.bass_utils import run_bass_kernel_spmd

BF16 = ml_dtypes.bfloat16
F32 = mybir.dt.float32
F32R = mybir.dt.float32r
BF = mybir.dt.bfloat16

N_CORES = 8
B, L, D = 2, 2048, 32
H = 8
NC16 = 16

_compiled_nc = None
LAST_RESULT = None


def _build():
    nc = bacc.Bacc("TRN2", target_bir_lowering=False, debug=False,
                   num_devices=N_CORES)

    xT_d = nc.declare_dram_parameter("xT", [32, L], F32R, isOutput=False)
    maskT_d = nc.declare_dram_parameter("maskT", [L, L], BF, isOutput=False)
    wcat_d = nc.declare_dram_parameter("wcat", [32, 384], F32R, isOutput=False)
    cp2_d = nc.declare_dram_parameter("cp2", [128, 512], BF, isOutput=False)
    wex_d = nc.declare_dram_parameter("wex", [128, 384], BF, isOutput=False)
    rsel_d = nc.declare_dram_parameter("rsel", [32, 1024], BF, isOutput=False)
    out_d = nc.declare_dram_parameter("out", [NC16, 128, 32], F32, isOutput=True)

    rs_dram = nc.dram_tensor("rs_bounce", [2, L], F32)
    qT_dram = nc.dram_tensor("qT_bounce", [2, 32, L], BF)
    uT_dram = nc.dram_tensor("uT_bounce", [2, 32, L], BF)

    with tile.TileContext(nc) as tc:
        with (
            tc.tile_pool(name="const", bufs=1) as cpool,
            tc.tile_pool(name="pt", bufs=5) as ptpool,
            tc.tile_pool(name="qrepa", bufs=1) as qrpoolA,
            tc.tile_pool(name="qhb", bufs=3) as qhpool,
            tc.tile_pool(name="urep", bufs=1) as urpool,
            tc.tile_pool(name="tbuf", bufs=3) as tpool,
            tc.tile_pool(name="small", bufs=2) as spool,
            tc.tile_pool(name="gpin", bufs=2) as gpool,
            tc.tile_pool(name="ps", bufs=1, space="PSUM") as pspool,
            tc.tile_pool(name="psw", bufs=2, space="PSUM") as pswork,
        ):
            # ---------- constants / inputs ----------
            wcat = cpool.tile([32, 384], F32R, tag="wcat")
            nc.sync.dma_start(out=wcat[:], in_=wcat_d[:])
            xT = cpool.tile([32, L], F32R, tag="xT")
            nc.sync.dma_start(out=xT[:, 0:1024], in_=xT_d[:, 0:1024])
            nc.sync.dma_start(out=xT[:, 1024:2048], in_=xT_d[:, 1024:2048])

            # whole mask resident; DMA spans hold their queue end-to-end, so
            # pairs alternate Pool/SP ordered to arrive just before phase B
            # consumes them. Act issues no DMAs (it paces the exp stream).
            maskT_sb = cpool.tile([128, NC16, L], BF, tag="mask")

            def load_mask_chunk(c):
                eng = nc.gpsimd if c % 2 == 0 else nc.sync
                eng.dma_start(
                    out=maskT_sb[:, c, :],
                    in_=maskT_d[128 * c:128 * c + 128, :],
                )

            for _c in range(4):
                load_mask_chunk(_c)
            cp_sb = cpool.tile([128, 8, 64], BF, tag="cp")
            wex = cpool.tile([128, 384], BF, tag="wex")
            woT2 = [wex[0:32, 0:32], wex[0:32, 192:224]]
            id25 = wex[:, 32:160]
            id1 = wex[0:32, 160:192]
            r4 = wex[0:32, 224:352]    # [I32 I32 I32 I32] selector

            qku = cpool.tile([32, 4 * L], F32R, tag="qku")
            qbf1 = cpool.tile([32, L], BF, tag="qbf1")   # h1 Q^T bf16
            ubf1 = cpool.tile([32, L], BF, tag="ubf1")   # h1 U bf16
            rsel = cpool.tile([32, 8, 128], BF, tag="rsel")
            proj_l = cpool.tile([128, NC16, 164], BF, tag="projl")
            rs_lp = cpool.tile([128, 32], F32, tag="rslp")
            invrs = cpool.tile([128, 32], F32, tag="invrs")
            nv25 = cpool.tile([128, 16], F32, tag="nv25")
            w2g = cpool.tile([128, NC16, 32], BF, tag="w2g")
            final_sb = cpool.tile([128, NC16, 32], F32, tag="final")

            nc.gpsimd.memset(proj_l[:, :, 32:33], 1.0)
            nc.gpsimd.memset(proj_l[:, :, 65:66], 1.0)

            # ---------- phase A helpers ----------
            # head h's Q/Kg projections. Head 0's qku copies ride Act (fast,
            # pipeline-critical); head 1's (emitted interleaved into B(h0))
            # and the bf16 qT8 copies ride the DVE.
            def emit_qk_proj(h, t, lh):
                ps_qk = pswork.tile([128, 1024], F32, tag="work")
                for nt in range(2):
                    nc.tensor.matmul(
                        ps_qk[0:32, 512 * nt:512 * nt + 512],
                        wcat[:, 64 * h + 32 * t:64 * h + 32 * t + 32],
                        xT[:, 1024 * lh + 512 * nt:1024 * lh + 512 * nt + 512],
                        start=True, stop=True,
                    )
                qsl = qku[:, L * (2 * h + t) + 1024 * lh:
                          L * (2 * h + t) + 1024 * lh + 1024]
                if h == 0 and t == 0:
                    nc.scalar.copy(out=qsl, in_=ps_qk[0:32, :])
                else:
                    nc.vector.tensor_copy(out=qsl, in_=ps_qk[0:32, :])

            def emit_qT8(h):
                # bf16 Q^T staging: head 0 bounces via DRAM for the broadcast
                # DMA; head 1 keeps it in SBUF for PE-selector replication.
                if h == 1:
                    nc.vector.tensor_copy(out=qbf1[:], in_=qku[:, 2 * L:3 * L])
                    return
                for lh in range(2):
                    qT8h = gpool.tile([32, 1024], BF, tag="gpin")
                    nc.vector.tensor_copy(
                        out=qT8h[:],
                        in_=qku[:, 2 * L * h + 1024 * lh:
                                2 * L * h + 1024 * lh + 1024])
                    nc.gpsimd.dma_start(
                        out=qT_dram[h][:, 1024 * lh:1024 * lh + 1024],
                        in_=qT8h[:])

            def emit_vk_proj(c, on_act=False):
                ps_vk = pswork.tile([128, 1024], F32, tag="work")
                nc.tensor.matmul(
                    ps_vk[:, 0:256],
                    xT[:, 128 * c:128 * c + 128],
                    wcat[:, 128:384],
                    start=True, stop=True,
                )
                vdst = proj_l[:, c, 0:66].rearrange("p (a b) -> p a b", a=2)[:, :, 0:32]
                vsrc = ps_vk[:, 0:64].rearrange("p (a b) -> p a b", a=2)
                if on_act:
                    nc.scalar.copy(out=vdst, in_=vsrc)
                    nc.scalar.copy(out=proj_l[:, c, 66:162], in_=ps_vk[:, 64:160])
                else:
                    nc.vector.tensor_copy(out=vdst, in_=vsrc)
                    nc.vector.tensor_copy(out=proj_l[:, c, 66:162],
                                          in_=ps_vk[:, 64:160])

            def load_qrep(h, pool, engines):
                """Full [128, 8, L] replication tile for head h."""
                qh = pool.tile([128, 8, L], BF, tag="qrep")
                qv4 = qT_dram[h].rearrange("(a i) l -> i a l", a=8)
                for i in range(4):
                    engines[i % len(engines)].dma_start(
                        out=qh[32 * i:32 * i + 32, :, :],
                        in_=qv4[i].unsqueeze(0).to_broadcast([32, 8, L]),
                    )
                return qh

            # ---------- phase A: head 0 projections only ----------
            for lh in range(2):
                for t in range(2):
                    emit_qk_proj(0, t, lh)
            emit_qT8(0)
            for _c in range(4, 8):
                load_mask_chunk(_c)
            for c in range(3):
                emit_vk_proj(c, on_act=True)

            ps_vu = pspool.tile([128, L], F32, tag="vu")

            # ---------- emission helpers ----------
            # S+exp for chunk c and mask+PV for chunk c-1 are emitted
            # staggered so the PE queue never parks an S matmul behind a PV
            # that waits on the DVE mask multiply (dependency-cycle pacing).
            def emit_s_exp(h, c):
                qw = 2 * L * h
                pt = ptpool.tile([128, L], BF, tag="pt")
                for lh in range(2):
                    ps_s = pswork.tile([128, 1024], F32, tag="work")
                    for nt in range(2):
                        nc.tensor.matmul(
                            ps_s[:, 512 * nt:512 * nt + 512],
                            qku[:, qw + L + 128 * c:qw + L + 128 * c + 128],
                            qku[:, qw + 1024 * lh + 512 * nt:
                                qw + 1024 * lh + 512 * nt + 512],
                            start=True, stop=True,
                        )
                    nc.scalar.activation(
                        pt[:, 1024 * lh:1024 * lh + 1024], ps_s[:],
                        mybir.ActivationFunctionType.Exp,
                    )
                return pt

            def emit_warm(n):
                # Keep-warm matmuls into unused ps_vu partitions 33..63: the
                # PE clock ramps to full speed only after ~3us of continuous
                # busy, so bridging its sub-us idle gaps doubles S/PV speed.
                # scratch region: h1's PV rows, free for the whole of B(h0);
                # PV(h1, c0)'s start=True reset clears the garbage.
                for i in range(n):
                    nc.tensor.matmul(
                        ps_vu[64:94, 512 * (i % 4):512 * (i % 4) + 512],
                        id25[0:32, 0:30], maskT_sb[0:32, 0, 0:512],
                        start=True, stop=True, skip_group_check=True,
                    )

            def emit_mask_pv(h, c, pt, warm=0):
                nc.vector.tensor_tensor(
                    out=pt[:], in0=pt[:], in1=maskT_sb[:, c, :],
                    op=mybir.AluOpType.mult,
                )
                for nt in range(4):
                    nc.tensor.matmul(
                        ps_vu[64 * h:64 * h + 33, 512 * nt:512 * nt + 512],
                        proj_l[:, c, 33 * h:33 * h + 33],
                        pt[:, 512 * nt:512 * nt + 512],
                        start=(c == 0), stop=(c == NC16 - 1),
                        skip_group_check=True,
                    )
                emit_warm(warm)

            def phase_c_steps(h, qrep):
                """Emit head-h tail as a list of thunks (interleavable)."""
                v0 = 64 * h
                state = {}

                def s_rs_half(lh):
                    def f():
                        rs_seq = ptpool.tile([1, 1024], F32, tag="pt")
                        nc.scalar.copy(
                            out=rs_seq[:],
                            in_=ps_vu[v0 + 32:v0 + 33, 1024 * lh:1024 * lh + 1024])
                        nc.sync.dma_start(
                            out=rs_dram[h][1024 * lh:1024 * lh + 1024].unsqueeze(0),
                            in_=rs_seq[:, :])
                        col = 16 * h + 8 * lh
                        nc.sync.dma_start(
                            out=rs_lp[:, col:col + 8],
                            in_=rs_dram[h][1024 * lh:1024 * lh + 1024]
                                .rearrange("(c p) -> p c", p=128),
                        )
                        tmp16 = spool.tile([128, 8], F32, tag="tmp16")
                        nc.vector.tensor_scalar(tmp16[:], rs_lp[:, col:col + 8],
                                                1e-30, None,
                                                op0=mybir.AluOpType.add)
                        nc.vector.reciprocal(invrs[:, col:col + 8], tmp16[:])
                        if h == 0:
                            nc.vector.tensor_scalar(
                                nv25[:, 8 * lh:8 * lh + 8], rs_lp[:, col:col + 8],
                                0.0, -0.25, op0=mybir.AluOpType.is_gt,
                                op1=mybir.AluOpType.mult)
                            for gl in range(8 * lh, 8 * lh + 8):
                                nc.vector.tensor_scalar(
                                    w2g[:, gl, :], proj_l[:, gl, 130:162],
                                    nv25[:, gl:gl + 1], None,
                                    op0=mybir.AluOpType.mult)
                    return f

                def s_ucorr(lh):
                    def f():
                        for c in range(8 * lh, 8 * lh + 8):
                            kwin = proj_l[:, c, 66 + 32 * h:98 + 32 * h]
                            nc.vector.tensor_scalar(
                                kwin, kwin,
                                rs_lp[:, 16 * h + c:16 * h + c + 1], None,
                                op0=mybir.AluOpType.mult)
                            nc.tensor.matmul(
                                ps_vu[v0:v0 + 32, 128 * c:128 * c + 128],
                                kwin, id25,
                                start=False, stop=True, skip_group_check=True,
                            )
                    return f

                def s_uv(lh):
                    # h0: U lands in row-group 0 of its replication tile and
                    # bounces via DRAM (overlapped under B(h1)); h1: U goes to
                    # a bf16 strip for PE-selector replication in the tail.
                    def f():
                        if "urep" not in state:
                            urep_t = urpool.tile([128, L], BF, tag="urep")
                            state["urep"] = urep_t
                        urep = state["urep"]
                        sl = slice(1024 * lh, 1024 * lh + 1024)
                        if h == 0:
                            nc.vector.tensor_copy(out=urep[0:32, sl],
                                                  in_=ps_vu[v0:v0 + 32, sl])
                            nc.sync.dma_start(out=uT_dram[h][:, sl],
                                              in_=urep[0:32, sl])
                        else:
                            nc.vector.tensor_copy(out=ubf1[:, sl],
                                                  in_=ps_vu[v0:v0 + 32, sl])
                    return f

                def s_urep(lh, engines):
                    def f():
                        urep = state["urep"]
                        sl = slice(1024 * lh, 1024 * lh + 1024)
                        if h == 0:
                            for r in range(1, 4):
                                engines[r % len(engines)].dma_start(
                                    out=urep[32 * r:32 * r + 32, sl],
                                    in_=uT_dram[h][:, sl],
                                )
                        else:
                            ps_u = pswork.tile([128, 1024], F32, tag="work")
                            for nt in range(2):
                                nc.tensor.matmul(
                                    ps_u[:, 512 * nt:512 * nt + 512], r4,
                                    ubf1[:, 1024 * lh + 512 * nt:
                                         1024 * lh + 512 * nt + 512],
                                    start=True, stop=True)
                            nc.scalar.copy(out=urep[:, sl], in_=ps_u[:])
                    return f

                def s_gpstart(hf):
                    # gp accumulators live in ps_vu rows [v0, v0+32) freed by s_uv
                    def f():
                        usrc = state["urep"] if h == 0 else ubf1
                        for nt in range(2):
                            sl = slice(1024 * hf + 512 * nt,
                                       1024 * hf + 512 * nt + 512)
                            nc.tensor.matmul(
                                ps_vu[v0:v0 + 32, sl],
                                woT2[h], usrc[0:32, sl],
                                start=True, stop=False, skip_group_check=True,
                            )
                    return f

                def s_ta(a, halves=(0, 1)):
                    def f():
                        urep = state["urep"]
                        for hf in halves:
                            if qrep is not None:
                                qh = qrep[:, a, 1024 * hf:1024 * hf + 1024]
                            else:
                                # PE selector replication; alternate units
                                # between an Act psum->sbuf copy and a direct
                                # PSUM read in the DVE multiply so neither
                                # engine paces the whole stream
                                ps_q = pswork.tile([128, 1024], F32, tag="work")
                                for nt in range(2):
                                    nc.tensor.matmul(
                                        ps_q[:, 512 * nt:512 * nt + 512],
                                        rsel[:, a, :],
                                        qbf1[:, 1024 * hf + 512 * nt:
                                             1024 * hf + 512 * nt + 512],
                                        start=True, stop=True,
                                    )
                                qh_t = qhpool.tile([128, 1024], BF, tag="qhb")
                                nc.scalar.copy(out=qh_t[:], in_=ps_q[:])
                                qh = qh_t[:]
                            th = tpool.tile([128, 1024], BF, tag="tt")
                            nc.vector.tensor_tensor(
                                out=th[:], in0=qh,
                                in1=urep[:, 1024 * hf:1024 * hf + 1024],
                                op=mybir.AluOpType.mult)
                            for nt in range(2):
                                sl = slice(1024 * hf + 512 * nt,
                                           1024 * hf + 512 * nt + 512)
                                nc.tensor.matmul(
                                    ps_vu[v0:v0 + 32, sl],
                                    cp_sb[:, a, 32 * h:32 * h + 32],
                                    th[:, 512 * nt:512 * nt + 512],
                                    start=False, stop=(a == 7),
                                    skip_group_check=True,
                                )
                    return f

                def s_tail(hf):
                    def f():
                        gp_in = gpool.tile([32, 1024], BF, tag="gpin")
                        nc.vector.tensor_copy(
                            out=gp_in[:],
                            in_=ps_vu[v0:v0 + 32, 1024 * hf:1024 * hf + 1024])
                        ps_tr = pswork.tile([128, 256], BF, tag="work")
                        for lt in range(8):
                            nc.tensor.transpose(
                                out=ps_tr[:, 32 * lt:32 * lt + 32],
                                in_=gp_in[:, 128 * lt:128 * lt + 128],
                                identity=id1,
                            )
                        for lt in range(8):
                            gl = 8 * hf + lt
                            in1 = w2g[:, gl, :] if h == 0 else final_sb[:, gl, :]
                            nc.vector.scalar_tensor_tensor(
                                out=final_sb[:, gl, :],
                                in0=ps_tr[:, 32 * lt:32 * lt + 32],
                                scalar=invrs[:, 16 * h + gl:16 * h + gl + 1],
                                in1=in1,
                                op0=mybir.AluOpType.mult,
                                op1=mybir.AluOpType.add)
                    return f

                ur_engines = [nc.sync] if h == 0 else [nc.sync, nc.gpsimd]
                if h == 0:
                    steps = [s_rs_half(0), s_rs_half(1),
                             s_ucorr(0), s_uv(0), s_urep(0, ur_engines),
                             s_gpstart(0),
                             s_ucorr(1), s_uv(1), s_urep(1, ur_engines),
                             s_gpstart(1)]
                    steps += [s_ta(a) for a in range(8)]
                    steps += [s_tail(0), s_tail(1)]
                else:
                    steps = [s_rs_half(0), s_rs_half(1),
                             s_ucorr(0), s_uv(0), s_urep(0, ur_engines),
                             s_gpstart(0),
                             s_ucorr(1), s_uv(1), s_urep(1, ur_engines),
                             s_gpstart(1)]
                    steps += [s_ta(a) for a in range(8)]
                    steps += [s_tail(0), s_tail(1)]
                return steps

            # ---------- head 0 main, with interleaved h1 projections ----------
            qrep0 = qrep1 = None
            h1_proj_units = [(t, lh) for lh in range(2) for t in range(2)]
            pt_prev = None
            for c in range(NC16):
                pt_c = emit_s_exp(0, c)
                if pt_prev is not None:
                    emit_mask_pv(0, c - 1, pt_prev, warm=0)
                pt_prev = pt_c
                if c <= 3:
                    load_mask_chunk(8 + 2 * c)
                    load_mask_chunk(9 + 2 * c)
                if c in (0, 2, 4, 6):
                    t, lh = h1_proj_units[c // 2]
                    emit_qk_proj(1, t, lh)
                elif c == 7:
                    emit_qT8(1)
                elif c == 5:
                    nc.sync.dma_start(
                        out=cp_sb[:].rearrange("p a c -> p (a c)"), in_=cp2_d[:])
                    nc.sync.dma_start(out=wex[:], in_=wex_d[:])
                elif c == 8:
                    qrep0 = load_qrep(0, qrpoolA, [nc.gpsimd])
                elif c == 9:
                    nc.sync.dma_start(
                        out=rsel[:].rearrange("p a c -> p (a c)"), in_=rsel_d[:])
                if 2 <= c < 15:
                    emit_vk_proj(c + 1, on_act=True)
            emit_mask_pv(0, NC16 - 1, pt_prev)

            # ---------- head 0 tail interleaved with head 1 main ----------
            steps0 = phase_c_steps(0, qrep0)
            pt_prev = None
            for c in range(NC16):
                pt_c = emit_s_exp(1, c)
                if pt_prev is not None:
                    emit_mask_pv(1, c - 1, pt_prev)
                pt_prev = pt_c
                if c < len(steps0):
                    steps0[c]()
            emit_mask_pv(1, NC16 - 1, pt_prev)
            for s in steps0[NC16:]:
                s()

            # ---------- head 1 tail ----------
            steps1 = phase_c_steps(1, None)
            for i, s in enumerate(steps1):
                s()
                if i == len(steps1) - 2:   # right after s_tail(0)
                    nc.sync.dma_start(
                        out=out_d[0:8].rearrange("c p d -> p c d"),
                        in_=final_sb[:, 0:8, :])
            nc.sync.dma_start(
                out=out_d[8:16].rearrange("c p d -> p c d"),
                in_=final_sb[:, 8:16, :])

    nc.compile()
    return nc


def _get_nc():
    global _compiled_nc
    if _compiled_nc is None:
        _compiled_nc = _build()
    return _compiled_nc


def _in_maps(x, mask, Wq, Wk, Wv, Wo, cayley, gs):
    s = 1.0 / math.sqrt(D)
    in_maps = []
    for core in range(N_CORES):
        b, hp = core // 4, core % 4
        heads = (2 * hp, 2 * hp + 1)
        xT = np.ascontiguousarray(x[b].T)
        maskT = np.ascontiguousarray(mask[b].T).astype(BF16)

        wcat = np.zeros((32, 384), np.float32)
        cp = np.zeros((1024, 64), np.float32)
        wex = np.zeros((128, 384), np.float32)
        W2sum = np.zeros((32, 32), np.float32)
        for j, h in enumerate(heads):
            Wq_h = Wq[32 * h:32 * h + 32]
            Wk_h = Wk[32 * h:32 * h + 32]
            Wv_h = Wv[32 * h:32 * h + 32]
            Wo_h = Wo[:, 32 * h:32 * h + 32]
            wcat[:, 64 * j:64 * j + 32] = Wq_h.T * s
            wcat[:, 64 * j + 32:64 * j + 64] = Wk_h.T * gs[None, :]
            wcat[:, 128 + 32 * j:128 + 32 * j + 32] = Wv_h.T
            wcat[:, 192 + 32 * j:192 + 32 * j + 32] = Wk_h.T
            W2sum += Wk_h.T @ Wo_h.T
            cp[:, 32 * j:32 * j + 32] = (
                math.sqrt(D) * np.einsum('ijk,dk->ijd', cayley, Wo_h)
            ).reshape(1024, 32)
            wex[0:32, (0 if j == 0 else 192):(32 if j == 0 else 224)] = Wo_h.T
        wcat[:, 256:288] = W2sum
        wex[:, 32:160] = 0.25 * np.eye(128)
        wex[0:32, 160:192] = np.eye(32)
        wex[0:32, 224:352] = np.tile(np.eye(32), (1, 4))
        cp2 = np.ascontiguousarray(
            cp.reshape(8, 128, 64).transpose(1, 0, 2).reshape(128, 512))

        # selector for PE replication: rsel[q, 128a + 32i + j] = (q == 4a+i)
        rsel = np.zeros((32, 8, 4, 32), np.float32)
        for a in range(8):
            for i in range(4):
                rsel[4 * a + i, a, i, :] = 1.0
        rsel = rsel.reshape(32, 1024)

        in_maps.append({
            "xT": xT,
            "maskT": maskT,
            "wcat": wcat,
            "cp2": cp2.astype(BF16),
            "wex": wex.astype(BF16),
            "rsel": rsel.astype(BF16),
        })
    return in_maps


def kernel(x, mask, Wq, Wk, Wv, Wo, cayley, grade_signs):
    x = np.asarray(x, dtype=np.float32)
    mask = np.asarray(mask)
    Wq = np.asarray(Wq, dtype=np.float32)
    Wk = np.asarray(Wk, dtype=np.float32)
    Wv = np.asarray(Wv, dtype=np.float32)
    Wo = np.asarray(Wo, dtype=np.float32)
    cayley = np.asarray(cayley, dtype=np.float32)
    gs = np.asarray(grade_signs, dtype=np.float32)

    in_maps = _in_maps(x, mask, Wq, Wk, Wv, Wo, cayley, gs)

    _trace = bool(os.environ.get("KTRACE"))
    res = run_bass_kernel_spmd(_get_nc(), in_maps, list(range(N_CORES)),
                               trace=_trace)
    global LAST_RESULT
    LAST_RESULT = res
    out = np.zeros((B, L, D), np.float32)
    for core in range(N_CORES):
        out[core // 4] += res.results[core]["out"].reshape(L, 32)
    return out
